# revision 13
# baseline (speedup 1.0000x reference)
"""Trainium2 Bass kernel for nn_Attention_17489106830121.

Math: the reference's einsums sum out entire axes, making attention logits
rank-1: attn[b,h,n,j] = s[b,n,h] * ks[b,j], with
  s  = x @ wqs              (wqs folds Wq head-colsums * SCALE * lksum)
  ks = LN(conv(x)) @ wk     (wk = even-col sums of Wkv)
  vs = LN(conv(x)) @ wv     (odd-col sums)
  out[b,n,:] = (softmax-weighted vs means over j, per (n,h)) @ Mmat + bproj

Per core: one batch element (8 cores == B).

v2 layout: the attention middle runs with j (the 256 reduced positions) on
PSUM/SBUF partitions:
  PE:    X_T[j, (n,h)] = ks_j * s[n,h] - t[n,h] as rank-2 matmuls from
         flattened (s; t) rows; num/den reductions as matmuls with E-slices
         as weights and [vs | 1] as rhs; final y matmul.
  ScalarE: one big Exp per chunk (PSUM -> SBUF bf16) -- the throughput floor.
  DVE:   LayerNorm, t-row build, per-nb divisions.
  GPSIMD: transpose-scatter copies, psum->sbuf copies of wt/y.
"""

import numpy as np

B, N, C, HEADS, SR = 8, 4096, 256, 8, 4
HC = C // HEADS          # 32
SCALE = HC ** -0.5
EPS = 1e-5
HS = 64 // SR            # 16
N2 = HS * HS             # 256
NB = N // 128            # 32 row tiles

_NC_CACHE = {}


def _build_nc():
    import concourse.bass as bass
    import concourse.bacc as bacc
    import concourse.mybir as mybir
    from concourse import tile

    dt = mybir.dt
    f32, bf16 = dt.float32, dt.bfloat16
    AF = mybir.ActivationFunctionType
    ALU = mybir.AluOpType
    AX = mybir.AxisListType

    nc = bacc.Bacc(None, target_bir_lowering=False)

    x_d = nc.dram_tensor("x", [N, C], f32, kind="ExternalInput")
    ws_d = nc.dram_tensor("wsr", [SR * SR * C, C], f32, kind="ExternalInput")
    wqs_d = nc.dram_tensor("wqs", [C, HEADS], f32, kind="ExternalInput")
    wkv_d = nc.dram_tensor("wkv2", [C, 2], f32, kind="ExternalInput")
    mm_d = nc.dram_tensor("mmat", [HEADS, C], f32, kind="ExternalInput")
    br_d = nc.dram_tensor("brow", [4, C], f32, kind="ExternalInput")
    id_d = nc.dram_tensor("ident", [128, 128], f32, kind="ExternalInput")
    y_d = nc.dram_tensor("y", [N, C], f32, kind="ExternalOutput")

    NG = 8            # flatten groups (4 nb each)
    GN = N // NG      # 512 n per group

    with tile.TileContext(nc) as tc:
        with tc.tile_pool(name="const", bufs=1) as cp:
            xT = cp.tile([128, 2, N], f32)         # x transposed, c on partitions
            sT_sb = cp.tile([8, N], f32)           # s transposed, h on partitions
            tT_sb = cp.tile([8, N], f32)           # shift rows
            wqssb = cp.tile([128, 2, HEADS], f32)
            wkvsb = cp.tile([128, 2, 2], f32)
            mmsb = cp.tile([HEADS, C], f32)
            bsr_r = cp.tile([1, C], f32)
            gam_r = cp.tile([1, C], f32)
            bet_r = cp.tile([1, C], f32)
            bpr_r = cp.tile([1, C], f32)
            idsb = cp.tile([128, 128], f32)
            ones_row = cp.tile([1, 128], f32)
            eps_sb = cp.tile([128, 1], f32)
            xcv = cp.tile([128, 2, N2], f32)
            xm = cp.tile([128, 2, N2], f32)
            xn = cp.tile([128, 2, N2], f32)
            xnT = cp.tile([128, 2, N2], f32)
            ks_r = cp.tile([1, N2], f32)
            vs_r = cp.tile([1, N2], f32)
            bsrep = cp.tile([128, N2], f32)
            garep = cp.tile([128, N2], f32)
            berep = cp.tile([128, N2], f32)
            kmx = cp.tile([1, 1], f32)
            kmn = cp.tile([1, 1], f32)
            kmax8 = cp.tile([8, 1], f32)
            kmin8 = cp.tile([8, 1], f32)
            tmp8a = cp.tile([8, GN], f32)
            tmp8b = cp.tile([8, GN], f32)
            mu = cp.tile([128, 2], f32)
            var = cp.tile([128, 2], f32)
            logv = cp.tile([128, 2], f32)
            rstd = cp.tile([128, 2], f32)
            sq = cp.tile([128, N2], f32)
            ksp0 = cp.tile([2, 128], f32)          # [ks chunk; -1] lhsT
            ksp1 = cp.tile([2, 128], f32)
            vo0 = cp.tile([128, 2], bf16)          # [vs | 1] rhs
            vo1 = cp.tile([128, 2], bf16)
            nd_sb = cp.tile([128, 16], f32)
            dinv = cp.tile([128, HEADS], f32)
            wvc = cp.tile([128, NB, HEADS], f32)

            nc.sync.dma_start(wqssb[:], wqs_d[:].rearrange("(t p) h -> p t h", p=128))
            nc.sync.dma_start(wkvsb[:], wkv_d[:].rearrange("(t p) h -> p t h", p=128))
            nc.sync.dma_start(mmsb[:], mm_d[:])
            nc.sync.dma_start(bsr_r[:], br_d[0:1, :])
            nc.sync.dma_start(gam_r[:], br_d[1:2, :])
            nc.sync.dma_start(bet_r[:], br_d[2:3, :])
            nc.sync.dma_start(bpr_r[:], br_d[3:4, :])
            nc.sync.dma_start(idsb[:], id_d[:])
            nc.vector.memset(ones_row[:], 1.0)
            nc.vector.memset(eps_sb[:], EPS)
            nc.vector.memset(ksp0[:], -1.0)
            nc.vector.memset(ksp1[:], -1.0)
            nc.vector.memset(vo0[:, 1:2], 1.0)
            nc.vector.memset(vo1[:, 1:2], 1.0)

            # ---------------- Phase A/B/C: transposes, sT, conv, LN, ks/vs ---
            with (
                tc.tile_pool(name="xsp", bufs=1) as xsp,
                tc.tile_pool(name="wsp", bufs=1) as wsp,
            ):
                xs = xsp.tile([128, NB, C], f32)
                wssb = wsp.tile([128, 32, C], f32)
                # x in 4 chunks so transposes can start early
                xs4 = xs[:].rearrange("p (f t) c -> p f t c", f=4)
                xd4 = x_d[:].rearrange("(f t p) c -> p f t c", p=128, f=4)
                for f in range(4):
                    nc.sync.dma_start(xs4[:, f, :, :], xd4[:, f, :, :])
                nc.sync.dma_start(wssb[:], ws_d[:].rearrange("(t p) c -> p t c", p=128))

                # transpose x: c onto partitions, n scattered into patch order
                with tc.tile_pool(name="psA", bufs=2, space="PSUM") as pA:
                    for g in range(32):
                        tp = pA.tile([128, 2, 128], f32)
                        ct, t0 = divmod(g * 2, NB)
                        for q in range(2):
                            nc.tensor.transpose(
                                tp[:, q, :], xs[:, t0 + q, 128 * ct:128 * (ct + 1)],
                                idsb[:],
                            )
                        v2 = xT[:, ct, :].rearrange(
                            "p (kh kw ph pw) -> p kh kw ph pw", kh=4, kw=4, ph=16, pw=16
                        )
                        dst = v2[:, :, :, t0 // 2, :].rearrange("p kh kw pw -> p kh pw kw")
                        srcv = tp[:].rearrange(
                            "p q i -> p (q i)"
                        ).rearrange("p (kha pw kw) -> p kha pw kw", kha=4, pw=16, kw=4)
                        eng = nc.vector.tensor_copy if g % 2 == 0 else nc.scalar.copy
                        eng(dst, srcv)

                    # sT = wqs^T @ xT  (h on partitions, n free, pos order)
                    for k in range(8):
                        sps = pA.tile([8, 512], f32)
                        for ct in range(2):
                            nc.tensor.matmul(
                                sps[:],
                                lhsT=wqssb[:, ct, :],
                                rhs=xT[:, ct, 512 * k:512 * (k + 1)],
                                start=(ct == 0),
                                stop=(ct == 1),
                            )
                        eng = nc.vector.tensor_copy if k % 2 == 0 else nc.scalar.copy
                        eng(sT_sb[:, 512 * k:512 * (k + 1)], sps[:])

                # conv (kernel=stride=4) + LayerNorm
                with tc.tile_pool(name="psB", bufs=2, space="PSUM") as pB:
                    brp = pB.tile([128, N2], f32)
                    nc.tensor.matmul(brp[:], lhsT=ones_row[:], rhs=bsr_r[:],
                                     start=True, stop=True)
                    nc.vector.tensor_copy(bsrep[:], brp[:])
                    grp = pB.tile([128, N2], f32)
                    nc.tensor.matmul(grp[:], lhsT=ones_row[:], rhs=gam_r[:],
                                     start=True, stop=True)
                    nc.vector.tensor_copy(garep[:], grp[:])
                    bep = pB.tile([128, N2], f32)
                    nc.tensor.matmul(bep[:], lhsT=ones_row[:], rhs=bet_r[:],
                                     start=True, stop=True)
                    nc.vector.tensor_copy(berep[:], bep[:])
                    for m in range(2):
                        cps = pB.tile([128, N2], f32)
                        for kh in range(4):
                            for kw in range(4):
                                for ct in range(2):
                                    kidx = kh * 8 + kw * 2 + ct
                                    base = (kh * 4 + kw) * 256 + 128 * m
                                    nc.tensor.matmul(
                                        cps[:],
                                        lhsT=xT[:, ct, base:base + 128],
                                        rhs=wssb[:, kidx, :],
                                        start=(kidx == 0),
                                        stop=(kidx == 31),
                                    )
                        nc.vector.tensor_tensor(xcv[:, m, :], cps[:], bsrep[:], ALU.add)

                    for m in range(2):
                        nc.vector.reduce_sum(mu[:, m:m + 1], xcv[:, m, :], axis=AX.X)
                        nc.vector.tensor_scalar(
                            mu[:, m:m + 1], mu[:, m:m + 1], 1.0 / N2, None, ALU.mult
                        )
                        nc.vector.tensor_scalar(
                            xm[:, m, :], xcv[:, m, :], mu[:, m:m + 1], None, ALU.subtract
                        )
                        nc.vector.tensor_tensor(sq[:], xm[:, m, :], xm[:, m, :],
                                                ALU.mult)
                        nc.vector.reduce_sum(var[:, m:m + 1], sq[:], axis=AX.X)
                        # rstd = exp(-0.5*ln(var/N2 + eps)); Ln+Exp share a table set
                        nc.scalar.activation(
                            logv[:, m:m + 1], var[:, m:m + 1], AF.Ln,
                            bias=eps_sb[:], scale=1.0 / N2,
                        )
                        nc.scalar.activation(
                            rstd[:, m:m + 1], logv[:, m:m + 1], AF.Exp, scale=-0.5
                        )
                        nc.vector.tensor_scalar(
                            xn[:, m, :], xm[:, m, :], rstd[:, m:m + 1], None, ALU.mult
                        )
                        nc.vector.tensor_tensor(xm[:, m, :], xn[:, m, :], garep[:], ALU.mult)
                        nc.vector.tensor_tensor(xn[:, m, :], xm[:, m, :], berep[:], ALU.add)

                # ks/vs rows, kmax/kmin, lhsT/rhs prep for the attention middle
                with tc.tile_pool(name="psC", bufs=1, space="PSUM") as pC:
                    tp2 = pC.tile([128, 4, 128], f32)
                    for t2 in range(2):
                        for ct in range(2):
                            nc.tensor.transpose(
                                tp2[:, t2 * 2 + ct, :],
                                xn[:, t2, 128 * ct:128 * (ct + 1)],
                                idsb[:],
                            )
                    for t2 in range(2):
                        for ct in range(2):
                            nc.vector.tensor_copy(
                                xnT[:, ct, 128 * t2:128 * (t2 + 1)], tp2[:, t2 * 2 + ct, :]
                            )
                    kps_k = pC.tile([1, N2], f32)
                    kps_v = pC.tile([1, N2], f32)
                    for ct in range(2):
                        nc.tensor.matmul(
                            kps_k[:], lhsT=wkvsb[:, ct, 0:1], rhs=xnT[:, ct, :],
                            start=(ct == 0), stop=(ct == 1),
                        )
                        nc.tensor.matmul(
                            kps_v[:], lhsT=wkvsb[:, ct, 1:2], rhs=xnT[:, ct, :],
                            start=(ct == 0), stop=(ct == 1),
                        )
                    nc.vector.tensor_copy(ks_r[:], kps_k[:])
                    nc.vector.tensor_copy(vs_r[:], kps_v[:])
                    nc.vector.reduce_max(kmx[:], ks_r[:], axis=AX.X)
                    nc.vector.tensor_reduce(kmn[:], ks_r[:], axis=AX.X, op=ALU.min)
                    r1 = pC.tile([8, 1], f32)
                    nc.tensor.matmul(r1[:], lhsT=ones_row[:, 0:8], rhs=kmx[:],
                                     start=True, stop=True)
                    nc.vector.tensor_copy(kmax8[:], r1[:])
                    r2 = pC.tile([8, 1], f32)
                    nc.tensor.matmul(r2[:], lhsT=ones_row[:, 0:8], rhs=kmn[:],
                                     start=True, stop=True)
                    nc.vector.tensor_copy(kmin8[:], r2[:])
                    # ksp rows 0: ks chunks (row 1 = -1 memset above)
                    nc.vector.tensor_copy(ksp0[0:1, :], ks_r[0:1, 0:128])
                    nc.vector.tensor_copy(ksp1[0:1, :], ks_r[0:1, 128:256])
                    # vo cols 0: vs as columns (col 1 = 1.0 memset above)
                    tp3 = pC.tile([128, 2], f32)
                    nc.tensor.transpose(tp3[:, 0:1], vs_r[0:1, 0:128], idsb[0:1, 0:1])
                    nc.tensor.transpose(tp3[:, 1:2], vs_r[0:1, 128:256], idsb[0:1, 0:1])
                    nc.vector.tensor_copy(vo0[:, 0:1], tp3[:, 0:1])
                    nc.vector.tensor_copy(vo1[:, 0:1], tp3[:, 1:2])

            # ---------------- Main: t rows, (s,t) flatten, X -> Exp -> num/den
            with (
                tc.tile_pool(name="stp", bufs=3) as stp,
                tc.tile_pool(name="ep", bufs=4) as ep,
                tc.tile_pool(name="wsq", bufs=2) as wsq,
                tc.tile_pool(name="ysq", bufs=2) as ysq,
                tc.tile_pool(name="pX", bufs=2, space="PSUM") as pX,
                tc.tile_pool(name="pND", bufs=2, space="PSUM") as pND,
                tc.tile_pool(name="pWT", bufs=1, space="PSUM") as pWT,
                tc.tile_pool(name="pY", bufs=1, space="PSUM") as pY,
            ):
                st_list = {}

                def issue_group(g):
                    gs, ge = GN * g, GN * (g + 1)
                    if g % 2 == 0:
                        nc.vector.tensor_scalar(tmp8a[:, 0:GN], sT_sb[:, gs:ge],
                                                kmin8[:], None, ALU.mult)
                        nc.vector.scalar_tensor_tensor(
                            tT_sb[:, gs:ge], sT_sb[:, gs:ge], kmax8[:],
                            tmp8a[:, 0:GN], ALU.mult, ALU.max,
                        )
                    else:
                        nc.gpsimd.tensor_scalar(tmp8b[:, 0:GN], sT_sb[:, gs:ge],
                                                kmin8[:], None, ALU.mult)
                        nc.gpsimd.tensor_scalar(tT_sb[:, gs:ge], sT_sb[:, gs:ge],
                                                kmax8[:], None, ALU.mult)
                        nc.vector.tensor_tensor(tT_sb[:, gs:ge], tT_sb[:, gs:ge],
                                                tmp8b[:, 0:GN], ALU.max)
                    ST = stp.tile([2, 8, 4, 128], f32, name=f"ST{g}", tag="ST")
                    # dest (h, nb_local, n') contiguous; src iterates (h, nb, n')
                    srcv_s = sT_sb[:, gs:ge].rearrange("p (nb n) -> p nb n", nb=4)
                    srcv_t = tT_sb[:, gs:ge].rearrange("p (nb n) -> p nb n", nb=4)
                    nc.sync.dma_start(ST[0:1], srcv_s)
                    nc.sync.dma_start(ST[1:2], srcv_t)
                    st_list[g] = ST

                issue_group(0)
                issue_group(1)

                e_list = [None, None]
                for c in range(2 * NB):
                    if c % 8 == 0:
                        g_next = c // 8 + 2
                        if g_next < NG:
                            issue_group(g_next)
                    nb, jt = divmod(c, 2)
                    ksp = ksp0 if jt == 0 else ksp1
                    Xp = pX.tile([128, 1024], f32)
                    g, nbl = divmod(nb, 4)
                    for k in range(2):
                        nc.tensor.matmul(
                            Xp[:, 512 * k:512 * (k + 1)],
                            lhsT=ksp[:],
                            rhs=st_list[g][:, 4 * k:4 * k + 4, nbl, :],
                            start=True, stop=True,
                        )
                    E = ep.tile([128, 1024], bf16)
                    nc.scalar.activation(E[:], Xp[:], AF.Exp)
                    e_list[jt] = E
                    if jt == 0:
                        continue
                    # num/den: contraction over j via E-slices as weights
                    ND = pND.tile([128, 16], f32)
                    for h in range(HEADS):
                        for j2 in range(2):
                            nc.tensor.matmul(
                                ND[:, 2 * h:2 * h + 2],
                                lhsT=e_list[j2][:, 128 * h:128 * (h + 1)],
                                rhs=(vo0 if j2 == 0 else vo1)[:],
                                start=(j2 == 0), stop=(j2 == 1),
                            )
                    nc.vector.tensor_copy(nd_sb[:], ND[:])
                    nd2 = nd_sb[:].rearrange("p (h two) -> p h two", two=2)
                    nc.vector.reciprocal(dinv[:], nd2[:, :, 1])
                    nc.vector.tensor_tensor(wvc[:, nb, :], nd2[:, :, 0], dinv[:],
                                            ALU.mult)
                    # final y for this nb
                    wt = pWT.tile([8, 128], f32)
                    nc.tensor.transpose(wt[:], wvc[:, nb, :], idsb[:])
                    wts = wsq.tile([8, 128], f32)
                    nc.vector.tensor_copy(wts[:], wt[:])
                    yp = pY.tile([128, C], f32)
                    nc.tensor.matmul(yp[:], lhsT=wts[:], rhs=mmsb[:],
                                     start=True, stop=False)
                    nc.tensor.matmul(yp[:], lhsT=ones_row[:], rhs=bpr_r[:],
                                     start=False, stop=True)
                    ysb = ysq.tile([128, C], f32)
                    nc.vector.tensor_copy(ysb[:], yp[:])
                    # pos-block nb: kh,kw = divmod(nb//2,4); ph in 8*(nb%2)+[0,8)
                    kh, kw = divmod(nb // 2, 4)
                    y5 = y_d[:].rearrange(
                        "(ph q pw r) c -> ph q pw r c", ph=16, q=4, pw=16, r=4
                    )
                    nc.sync.dma_start(
                        y5[8 * (nb % 2):8 * (nb % 2) + 8, kh, :, kw, :], ysb[:]
                    )

    nc.compile()
    return nc


def _host_precompute(Wq, Wkv, Wsr, bsr, gamma, beta, Wproj, bproj, k_learn, v_learn):
    lksum = k_learn.reshape(HEADS, HC).sum(1)
    wqs = (Wq.reshape(C, HEADS, HC).sum(2) * (SCALE * lksum)[None, :]).astype(np.float32)
    wkv2 = np.stack([Wkv[:, 0::2].sum(1), Wkv[:, 1::2].sum(1)], 1).astype(np.float32)
    lv = v_learn.reshape(HEADS, HC)
    # out rearrange 'b d n c -> b n (c d)': column index = ci*HEADS + h
    Mmat = np.zeros((HEADS, C), np.float32)
    for h in range(HEADS):
        Mmat[h] = lv[h] @ Wproj[h::HEADS]
    Wsr_flat = np.ascontiguousarray(
        Wsr.transpose(2, 3, 1, 0).reshape(SR * SR * C, C)
    ).astype(np.float32)
    brow = np.stack([bsr, gamma, beta, bproj]).astype(np.float32)
    ident = np.eye(128, dtype=np.float32)
    return dict(wsr=Wsr_flat, wqs=wqs, wkv2=wkv2, mmat=Mmat, brow=brow, ident=ident)


def kernel(**inputs):
    x = np.asarray(inputs["x"], np.float32)
    weights = _host_precompute(
        *[np.asarray(inputs[k], np.float32) for k in
          ("Wq", "Wkv", "Wsr", "bsr", "gamma", "beta", "Wproj", "bproj",
           "k_learn", "v_learn")]
    )
    if "nc" not in _NC_CACHE:
        _NC_CACHE["nc"] = _build_nc()
    nc = _NC_CACHE["nc"]
    in_maps = [
        {"x": np.ascontiguousarray(x[i]), **weights} for i in range(B)
    ]
    from concourse.bass_utils import run_bass_kernel_spmd

    res = run_bass_kernel_spmd(nc, in_maps, core_ids=list(range(B)))
    y = np.stack([res.results[i]["y"] for i in range(B)], 0)
    return y


# revision 20
# speedup vs baseline: 1.4284x; 1.4284x over previous
"""Trainium2 Bass kernel for nn_Attention_17489106830121.

Math: the reference's einsums sum out entire axes, making attention logits
rank-1: attn[b,h,n,j] = s[b,n,h] * ks[b,j], with
  s  = x @ wqs              (wqs folds Wq head-colsums * SCALE * lksum)
  ks = LN(conv(x)) @ wk     (wk = even-col sums of Wkv)
  vs = LN(conv(x)) @ wv     (odd-col sums)
  out[b,n,:] = (softmax-weighted vs means over j, per (n,h)) @ Mmat + bproj

Per core: one batch element (8 cores == B).

v3: j-on-partitions attention middle, minimal PE instruction count.
Shifted logits via sign-split (exact, no shift row):
  X[j, (n,h)] = (ks_j - kmax) * s+  +  (ks_j - kmin) * s-
with s+/s- and the two k-vectors in fp16 hi/lo pairs (K=6 rank per chunk,
error ~1e-4). One matmul per (nb, jt) builds X [128, 1024]; ScalarE Exp's
it psum->sbuf bf16; one [vs|1]-weighted matmul per jt reduces num/den to
[2, 1024] rows; DMA reshapes rows to [8, 128] head-major tiles where the
division runs on DVE; final y = wts.T @ Mmat + bias.
"""

import numpy as np

B, N, C, HEADS, SR = 8, 4096, 256, 8, 4
HC = C // HEADS          # 32
SCALE = HC ** -0.5
EPS = 1e-5
HS = 64 // SR            # 16
N2 = HS * HS             # 256
NB = N // 128            # 32 row tiles

_NC_CACHE = {}


def _build_nc():
    import concourse.bass as bass
    import concourse.bacc as bacc
    import concourse.mybir as mybir
    from concourse import tile

    dt = mybir.dt
    f32, bf16, f16 = dt.float32, dt.bfloat16, dt.float16
    AF = mybir.ActivationFunctionType
    ALU = mybir.AluOpType
    AX = mybir.AxisListType

    nc = bacc.Bacc(None, target_bir_lowering=False)

    x_d = nc.dram_tensor("x", [N, C], f32, kind="ExternalInput")
    ws_d = nc.dram_tensor("wsr", [SR * SR * C, C], f32, kind="ExternalInput")
    wqs_d = nc.dram_tensor("wqs", [C, HEADS], f32, kind="ExternalInput")
    wkv_d = nc.dram_tensor("wkv2", [C, 2], f32, kind="ExternalInput")
    mm_d = nc.dram_tensor("mmat", [HEADS, C], f32, kind="ExternalInput")
    br_d = nc.dram_tensor("brow", [4, C], f32, kind="ExternalInput")
    id_d = nc.dram_tensor("ident", [128, 128], f32, kind="ExternalInput")
    y_d = nc.dram_tensor("y", [N, C], f32, kind="ExternalOutput")

    NG = 4            # flatten groups (8 nb each)
    GNB = NB // NG    # 8 nb per group
    GN = N // NG      # 1024 n per group

    with tile.TileContext(nc) as tc:
        with tc.tile_pool(name="const", bufs=1) as cp:
            sphi = cp.tile([8, N], f16)            # fp16 hi/lo pairs of s+ / s-
            splo = cp.tile([8, N], f16)
            smhi = cp.tile([8, N], f16)
            smlo = cp.tile([8, N], f16)
            wts_all = cp.tile([8, NB, 128], f32)   # r = num/den, head-major
            wqssb = cp.tile([128, 2, HEADS], f32)
            wkvsb = cp.tile([128, 2, 2], f32)
            mmsb = cp.tile([HEADS, C], f32)
            bsr_r = cp.tile([1, C], f32)
            gam_r = cp.tile([1, C], f32)
            bet_r = cp.tile([1, C], f32)
            bpr_r = cp.tile([1, C], f32)
            idsb = cp.tile([128, 128], f32)
            ones_row = cp.tile([1, 128], f32)
            ones256 = cp.tile([1, N2], f32)
            eps_sb = cp.tile([128, 1], f32)
            xcv = cp.tile([128, 2, N2], f32)
            xm = cp.tile([128, 2, N2], f32)
            xn = cp.tile([128, 2, N2], f32)
            xnT = cp.tile([128, 2, N2], f32)
            ks_r = cp.tile([1, N2], f32)
            vs_r = cp.tile([1, N2], f32)
            kA_r = cp.tile([1, N2], f32)
            kB_r = cp.tile([1, N2], f32)
            khl16 = cp.tile([1, 4, N2], f16)       # fp16 hi/lo of kA, kB
            khl32 = cp.tile([1, 4, N2], f32)       # same values, f32 for transpose
            kmrow = cp.tile([1, 2, N2], f32)       # kmax row, kmin row
            kcols = cp.tile([128, 12], f32)
            lhsT6 = cp.tile([6, 2, 128], f16)
            bsrep = cp.tile([128, N2], f32)
            garep = cp.tile([128, N2], f32)
            berep = cp.tile([128, N2], f32)
            bprep = cp.tile([128, C], f32)
            kmx = cp.tile([1, 1], f32)
            kmn = cp.tile([1, 1], f32)
            mu = cp.tile([128, 2], f32)
            var = cp.tile([128, 2], f32)
            logv = cp.tile([128, 2], f32)
            rstd = cp.tile([128, 2], f32)
            sq = cp.tile([128, N2], f32)
            vo0 = cp.tile([128, 2], bf16)          # [vs | 1] reduction weights
            vo1 = cp.tile([128, 2], bf16)

            nc.sync.dma_start(wqssb[:], wqs_d[:].rearrange("(t p) h -> p t h", p=128))
            nc.sync.dma_start(wkvsb[:], wkv_d[:].rearrange("(t p) h -> p t h", p=128))
            nc.sync.dma_start(mmsb[:], mm_d[:])
            nc.sync.dma_start(bsr_r[:], br_d[0:1, :])
            nc.sync.dma_start(gam_r[:], br_d[1:2, :])
            nc.sync.dma_start(bet_r[:], br_d[2:3, :])
            nc.sync.dma_start(bpr_r[:], br_d[3:4, :])
            nc.sync.dma_start(idsb[:], id_d[:])
            nc.vector.memset(ones_row[:], 1.0)
            nc.vector.memset(ones256[:], 1.0)
            nc.vector.memset(eps_sb[:], EPS)
            nc.vector.memset(vo0[:, 1:2], 1.0)
            nc.vector.memset(vo1[:, 1:2], 1.0)

            # ---------------- Phase A/B/C: transposes, sT, conv, LN, ks/vs ---
            with tc.tile_pool(name="wsp", bufs=1) as wsp:
                wssb = wsp.tile([128, 32, C], f32)
                xT = wsp.tile([128, 2, N], f32)    # x transposed, c on partitions
                nc.sync.dma_start(wssb[:], ws_d[:].rearrange("(t p) c -> p t c", p=128))
                with tc.tile_pool(name="xsp1", bufs=1) as xsp1:
                    xs = xsp1.tile([128, NB, C], f32)
                    xs4 = xs[:].rearrange("p (f t) c -> p f t c", f=4)
                    xd4 = x_d[:].rearrange("(f t p) c -> p f t c", p=128, f=4)
                    for f in range(4):
                        nc.sync.dma_start(xs4[:, f, :, :], xd4[:, f, :, :])

                    with tc.tile_pool(name="psA", bufs=2, space="PSUM") as pA:
                        for g in range(32):
                            tp = pA.tile([128, 2, 128], f32)
                            ct, t0 = divmod(g * 2, NB)
                            for q in range(2):
                                nc.tensor.transpose(
                                    tp[:, q, :], xs[:, t0 + q, 128 * ct:128 * (ct + 1)],
                                    idsb[:],
                                )
                            v2 = xT[:, ct, :].rearrange(
                                "p (kh kw ph pw) -> p kh kw ph pw", kh=4, kw=4, ph=16, pw=16
                            )
                            dst = v2[:, :, :, t0 // 2, :].rearrange("p kh kw pw -> p kh pw kw")
                            srcv = tp[:].rearrange(
                                "p q i -> p (q i)"
                            ).rearrange("p (kha pw kw) -> p kha pw kw", kha=4, pw=16, kw=4)
                            eng = nc.vector.tensor_copy if g % 2 == 0 else nc.scalar.copy
                            eng(dst, srcv)

                with tc.tile_pool(name="xsp2", bufs=1) as xsp2:
                    sT_sb = xsp2.tile([8, N], f32)
                    zeros8 = xsp2.tile([8, N], f32)
                    scratch = xsp2.tile([8, N], f32)
                    nc.vector.memset(zeros8[:], 0.0)
                    with tc.tile_pool(name="psA2", bufs=2, space="PSUM") as pA2:
                        # sT = wqs^T @ xT  (h on partitions, n free, pos order)
                        for k in range(8):
                            sps = pA2.tile([8, 512], f32)
                            for ct in range(2):
                                nc.tensor.matmul(
                                    sps[:],
                                    lhsT=wqssb[:, ct, :],
                                    rhs=xT[:, ct, 512 * k:512 * (k + 1)],
                                    start=(ct == 0),
                                    stop=(ct == 1),
                                )
                            eng = nc.vector.tensor_copy if k % 2 == 0 else nc.scalar.copy
                            eng(sT_sb[:, 512 * k:512 * (k + 1)], sps[:])
                    # s+/s- fp16 hi/lo splits
                    nc.vector.tensor_tensor(scratch[:], sT_sb[:], zeros8[:], ALU.max)
                    nc.vector.tensor_copy(sphi[:], scratch[:])
                    nc.vector.tensor_tensor(splo[:], scratch[:], sphi[:], ALU.subtract)
                    nc.vector.tensor_tensor(scratch[:], sT_sb[:], zeros8[:], ALU.min)
                    nc.vector.tensor_copy(smhi[:], scratch[:])
                    nc.vector.tensor_tensor(smlo[:], scratch[:], smhi[:], ALU.subtract)

                # conv (kernel=stride=4) + LayerNorm
                with tc.tile_pool(name="psB", bufs=2, space="PSUM") as pB:
                    brp = pB.tile([128, N2], f32, bufs=1)
                    nc.tensor.matmul(brp[:], lhsT=ones_row[:], rhs=bsr_r[:],
                                     start=True, stop=True)
                    nc.vector.tensor_copy(bsrep[:], brp[:])
                    grp = pB.tile([128, N2], f32, bufs=1)
                    nc.tensor.matmul(grp[:], lhsT=ones_row[:], rhs=gam_r[:],
                                     start=True, stop=True)
                    nc.vector.tensor_copy(garep[:], grp[:])
                    bep = pB.tile([128, N2], f32, bufs=1)
                    nc.tensor.matmul(bep[:], lhsT=ones_row[:], rhs=bet_r[:],
                                     start=True, stop=True)
                    nc.vector.tensor_copy(berep[:], bep[:])
                    bpp = pB.tile([128, C], f32, bufs=1)
                    nc.tensor.matmul(bpp[:], lhsT=ones_row[:], rhs=bpr_r[:],
                                     start=True, stop=True)
                    nc.vector.tensor_copy(bprep[:], bpp[:])
                    for m in range(2):
                        cps = pB.tile([128, N2], f32)
                        for kh in range(4):
                            for kw in range(4):
                                for ct in range(2):
                                    kidx = kh * 8 + kw * 2 + ct
                                    base = (kh * 4 + kw) * 256 + 128 * m
                                    nc.tensor.matmul(
                                        cps[:],
                                        lhsT=xT[:, ct, base:base + 128],
                                        rhs=wssb[:, kidx, :],
                                        start=(kidx == 0),
                                        stop=(kidx == 31),
                                    )
                        nc.vector.tensor_tensor(xcv[:, m, :], cps[:], bsrep[:], ALU.add)

                    for m in range(2):
                        nc.vector.reduce_sum(mu[:, m:m + 1], xcv[:, m, :], axis=AX.X)
                        nc.vector.tensor_scalar(
                            mu[:, m:m + 1], mu[:, m:m + 1], 1.0 / N2, None, ALU.mult
                        )
                        nc.vector.tensor_scalar(
                            xm[:, m, :], xcv[:, m, :], mu[:, m:m + 1], None, ALU.subtract
                        )
                        nc.vector.tensor_tensor(sq[:], xm[:, m, :], xm[:, m, :],
                                                ALU.mult)
                        nc.vector.reduce_sum(var[:, m:m + 1], sq[:], axis=AX.X)
                        nc.scalar.activation(
                            logv[:, m:m + 1], var[:, m:m + 1], AF.Ln,
                            bias=eps_sb[:], scale=1.0 / N2,
                        )
                        nc.scalar.activation(
                            rstd[:, m:m + 1], logv[:, m:m + 1], AF.Exp, scale=-0.5
                        )
                        nc.vector.tensor_scalar(
                            xn[:, m, :], xm[:, m, :], rstd[:, m:m + 1], None, ALU.mult
                        )
                        nc.vector.tensor_tensor(xm[:, m, :], xn[:, m, :], garep[:], ALU.mult)
                        nc.vector.tensor_tensor(xn[:, m, :], xm[:, m, :], berep[:], ALU.add)

            # ks/vs rows, k split-precision prep
            with tc.tile_pool(name="psC", bufs=1, space="PSUM") as pC:
                tp2 = pC.tile([128, 4, 128], f32)
                for t2 in range(2):
                    for ct in range(2):
                        nc.tensor.transpose(
                            tp2[:, t2 * 2 + ct, :],
                            xn[:, t2, 128 * ct:128 * (ct + 1)],
                            idsb[:],
                        )
                for t2 in range(2):
                    for ct in range(2):
                        nc.vector.tensor_copy(
                            xnT[:, ct, 128 * t2:128 * (t2 + 1)], tp2[:, t2 * 2 + ct, :]
                        )
                kps_k = pC.tile([1, N2], f32)
                kps_v = pC.tile([1, N2], f32)
                for ct in range(2):
                    nc.tensor.matmul(
                        kps_k[:], lhsT=wkvsb[:, ct, 0:1], rhs=xnT[:, ct, :],
                        start=(ct == 0), stop=(ct == 1),
                    )
                    nc.tensor.matmul(
                        kps_v[:], lhsT=wkvsb[:, ct, 1:2], rhs=xnT[:, ct, :],
                        start=(ct == 0), stop=(ct == 1),
                    )
                nc.vector.tensor_copy(ks_r[:], kps_k[:])
                nc.vector.tensor_copy(vs_r[:], kps_v[:])
                nc.vector.reduce_max(kmx[:], ks_r[:], axis=AX.X)
                nc.vector.tensor_reduce(kmn[:], ks_r[:], axis=AX.X, op=ALU.min)
                # kmax/kmin broadcast rows; kA = ks - kmax, kB = ks - kmin
                kmp = pC.tile([1, 2, N2], f32)
                nc.tensor.matmul(kmp[:, 0, :], lhsT=kmx[:], rhs=ones256[:],
                                 start=True, stop=True)
                nc.tensor.matmul(kmp[:, 1, :], lhsT=kmn[:], rhs=ones256[:],
                                 start=True, stop=True)
                nc.vector.tensor_copy(kmrow[:], kmp[:])
                nc.vector.tensor_tensor(kA_r[:], ks_r[:], kmrow[:, 0, :], ALU.subtract)
                nc.vector.tensor_tensor(kB_r[:], ks_r[:], kmrow[:, 1, :], ALU.subtract)
                # fp16 hi/lo split of kA/kB, kept in f32 rows for PE transpose
                nc.vector.tensor_copy(khl16[:, 0, :], kA_r[:])
                nc.vector.tensor_copy(khl32[:, 0, :], khl16[:, 0, :])
                nc.vector.tensor_tensor(khl16[:, 1, :], kA_r[:], khl32[:, 0, :],
                                        ALU.subtract)
                nc.vector.tensor_copy(khl32[:, 1, :], khl16[:, 1, :])
                nc.vector.tensor_copy(khl16[:, 2, :], kB_r[:])
                nc.vector.tensor_copy(khl32[:, 2, :], khl16[:, 2, :])
                nc.vector.tensor_tensor(khl16[:, 3, :], kB_r[:], khl32[:, 2, :],
                                        ALU.subtract)
                nc.vector.tensor_copy(khl32[:, 3, :], khl16[:, 3, :])
                # pack k columns per jt with duplicates, transpose to lhsT6
                # lhsT6 rows: (kAhi, kAhi, kAlo, kBhi, kBhi, kBlo)
                kcp = pC.tile([128, 12], f32)
                for jt in range(2):
                    for ci, ki in ((0, 0), (1, 0), (2, 1), (3, 2), (4, 2), (5, 3)):
                        nc.tensor.transpose(
                            kcp[:, 6 * jt + ci:6 * jt + ci + 1],
                            khl32[0:1, ki, 128 * jt:128 * (jt + 1)], idsb[0:1, 0:1],
                        )
                nc.vector.tensor_copy(kcols[:], kcp[:])
                l6p = pC.tile([6, 2, 128], f32)
                for jt in range(2):
                    nc.tensor.transpose(l6p[:, jt, :], kcols[:, 6 * jt:6 * jt + 6],
                                        idsb[:])
                nc.vector.tensor_copy(lhsT6[:], l6p[:])
                # vs as columns
                tp3 = pC.tile([128, 2], f32)
                nc.tensor.transpose(tp3[:, 0:1], vs_r[0:1, 0:128], idsb[0:1, 0:1])
                nc.tensor.transpose(tp3[:, 1:2], vs_r[0:1, 128:256], idsb[0:1, 0:1])
                nc.vector.tensor_copy(vo0[:, 0:1], tp3[:, 0:1])
                nc.vector.tensor_copy(vo1[:, 0:1], tp3[:, 1:2])

            # ------------- Main: flatten rows, X -> Exp -> num/den -> wts
            if True:
                with (
                    tc.tile_pool(name="stp", bufs=2) as stp,
                    tc.tile_pool(name="ep", bufs=4) as ep,
                    tc.tile_pool(name="ndp", bufs=3) as ndp,
                    tc.tile_pool(name="ntp", bufs=2) as ntp,
                    tc.tile_pool(name="dtp", bufs=2) as dtp,
                    tc.tile_pool(name="dvp", bufs=2) as dvp,
                    tc.tile_pool(name="pX", bufs=2, space="PSUM") as pX,
                    tc.tile_pool(name="pND", bufs=2, space="PSUM") as pND,
                ):
                    st_list = {}

                    def issue_group(g):
                        gs, ge = GN * g, GN * (g + 1)
                        ST = stp.tile([6, 8, GNB, 128], f16, name=f"ST{g}", tag="ST")
                        for r, src in ((0, sphi), (1, splo), (2, sphi),
                                       (3, smhi), (4, smlo), (5, smhi)):
                            srcv = src[:, gs:ge].rearrange("p (nb n) -> p nb n", nb=GNB)
                            nc.sync.dma_start(ST[r:r + 1], srcv)
                        st_list[g] = ST

                    issue_group(0)

                    e_pair = [None, None]
                    for c in range(2 * NB):
                        nb, jt = divmod(c, 2)
                        g, nbl = divmod(nb, GNB)
                        if nbl == 0 and jt == 0 and g + 1 < NG:
                            issue_group(g + 1)
                        Xp = pX.tile([128, 1024], f32)
                        for k in range(2):
                            nc.tensor.matmul(
                                Xp[:, 512 * k:512 * (k + 1)],
                                lhsT=lhsT6[:, jt, :],
                                rhs=st_list[g][:, 4 * k:4 * k + 4, nbl, :],
                                start=True, stop=True,
                            )
                        E = ep.tile([128, 1024], bf16)
                        nc.scalar.activation(E[:], Xp[:], AF.Exp)
                        e_pair[jt] = E
                        if jt == 0:
                            continue
                        ND = pND.tile([2, 1024], f32)
                        for k in range(2):
                            sl = slice(512 * k, 512 * (k + 1))
                            nc.tensor.matmul(ND[:, sl], lhsT=vo0[:],
                                             rhs=e_pair[0][:, sl],
                                             start=True, stop=False)
                            nc.tensor.matmul(ND[:, sl], lhsT=vo1[:],
                                             rhs=e_pair[1][:, sl],
                                             start=False, stop=True)
                        nd_sb = ndp.tile([2, 1024], f32)
                        eng = nc.vector.tensor_copy if nb % 2 == 0 else nc.scalar.copy
                        eng(nd_sb[:], ND[:])
                        numT = ntp.tile([8, 128], f32)
                        denT = dtp.tile([8, 128], f32)
                        nc.sync.dma_start(
                            numT[:], nd_sb[0:1, :].rearrange("p (h n) -> p h n", h=8))
                        nc.sync.dma_start(
                            denT[:], nd_sb[1:2, :].rearrange("p (h n) -> p h n", h=8))
                        dinvT = dvp.tile([8, 128], f32)
                        nc.vector.reciprocal(dinvT[:], denT[:])
                        nc.vector.tensor_tensor(wts_all[:, nb, :], numT[:], dinvT[:],
                                                ALU.mult)

            # ---------------- Final: y = wts.T @ Mmat + bias ----------------
            with (
                tc.tile_pool(name="ysq", bufs=2) as ysq,
                tc.tile_pool(name="pY", bufs=2, space="PSUM") as pY,
            ):
                for nb in range(NB):
                    yp = pY.tile([128, C], f32)
                    nc.tensor.matmul(yp[:], lhsT=wts_all[:, nb, :], rhs=mmsb[:],
                                     start=True, stop=True)
                    ysb = ysq.tile([128, C], f32)
                    nc.vector.tensor_tensor(ysb[:], yp[:], bprep[:], ALU.add)
                    kh, kw = divmod(nb // 2, 4)
                    y5 = y_d[:].rearrange(
                        "(ph q pw r) c -> ph q pw r c", ph=16, q=4, pw=16, r=4
                    )
                    nc.sync.dma_start(
                        y5[8 * (nb % 2):8 * (nb % 2) + 8, kh, :, kw, :], ysb[:]
                    )

    nc.compile()
    return nc


def _host_precompute(Wq, Wkv, Wsr, bsr, gamma, beta, Wproj, bproj, k_learn, v_learn):
    lksum = k_learn.reshape(HEADS, HC).sum(1)
    wqs = (Wq.reshape(C, HEADS, HC).sum(2) * (SCALE * lksum)[None, :]).astype(np.float32)
    wkv2 = np.stack([Wkv[:, 0::2].sum(1), Wkv[:, 1::2].sum(1)], 1).astype(np.float32)
    lv = v_learn.reshape(HEADS, HC)
    # out rearrange 'b d n c -> b n (c d)': column index = ci*HEADS + h
    Mmat = np.zeros((HEADS, C), np.float32)
    for h in range(HEADS):
        Mmat[h] = lv[h] @ Wproj[h::HEADS]
    Wsr_flat = np.ascontiguousarray(
        Wsr.transpose(2, 3, 1, 0).reshape(SR * SR * C, C)
    ).astype(np.float32)
    brow = np.stack([bsr, gamma, beta, bproj]).astype(np.float32)
    ident = np.eye(128, dtype=np.float32)
    return dict(wsr=Wsr_flat, wqs=wqs, wkv2=wkv2, mmat=Mmat, brow=brow, ident=ident)


def kernel(**inputs):
    x = np.asarray(inputs["x"], np.float32)
    weights = _host_precompute(
        *[np.asarray(inputs[k], np.float32) for k in
          ("Wq", "Wkv", "Wsr", "bsr", "gamma", "beta", "Wproj", "bproj",
           "k_learn", "v_learn")]
    )
    if "nc" not in _NC_CACHE:
        _NC_CACHE["nc"] = _build_nc()
    nc = _NC_CACHE["nc"]
    in_maps = [
        {"x": np.ascontiguousarray(x[i]), **weights} for i in range(B)
    ]
    from concourse.bass_utils import run_bass_kernel_spmd

    res = run_bass_kernel_spmd(nc, in_maps, core_ids=list(range(B)))
    y = np.stack([res.results[i]["y"] for i in range(B)], 0)
    return y


# revision 23
# speedup vs baseline: 1.6511x; 1.1559x over previous
"""Trainium2 Bass kernel for nn_Attention_17489106830121.

Math: the reference's einsums sum out entire axes, making attention logits
rank-1: attn[b,h,n,j] = s[b,n,h] * ks[b,j], with
  s  = x @ wqs              (wqs folds Wq head-colsums * SCALE * lksum)
  ks = LN(conv(x)) @ wk     (wk = even-col sums of Wkv)
  vs = LN(conv(x)) @ wv     (odd-col sums)
  out[b,n,:] = (softmax-weighted vs means over j, per (n,h)) @ Mmat + bproj

Per core: one batch element (8 cores == B).

v3: j-on-partitions attention middle, minimal PE instruction count.
Shifted logits via sign-split (exact, no shift row):
  X[j, (n,h)] = (ks_j - kmax) * s+  +  (ks_j - kmin) * s-
with s+/s- and the two k-vectors in fp16 hi/lo pairs (K=6 rank per chunk,
error ~1e-4). One matmul per (nb, jt) builds X [128, 1024]; ScalarE Exp's
it psum->sbuf bf16; one [vs|1]-weighted matmul per jt reduces num/den to
[2, 1024] rows; DMA reshapes rows to [8, 128] head-major tiles where the
division runs on DVE; final y = wts.T @ Mmat + bias.
"""

import numpy as np

B, N, C, HEADS, SR = 8, 4096, 256, 8, 4
HC = C // HEADS          # 32
SCALE = HC ** -0.5
EPS = 1e-5
HS = 64 // SR            # 16
N2 = HS * HS             # 256
NB = N // 128            # 32 row tiles

_NC_CACHE = {}


def _build_nc():
    import concourse.bass as bass
    import concourse.bacc as bacc
    import concourse.mybir as mybir
    from concourse import tile

    dt = mybir.dt
    f32, bf16, f16 = dt.float32, dt.bfloat16, dt.float16
    AF = mybir.ActivationFunctionType
    ALU = mybir.AluOpType
    AX = mybir.AxisListType

    nc = bacc.Bacc(None, target_bir_lowering=False)

    x_d = nc.dram_tensor("x", [N, C], f32, kind="ExternalInput")
    ws_d = nc.dram_tensor("wsr", [SR * SR * C, C], f32, kind="ExternalInput")
    wqs_d = nc.dram_tensor("wqs", [C, HEADS], f32, kind="ExternalInput")
    wkv_d = nc.dram_tensor("wkv2", [C, 2], f32, kind="ExternalInput")
    mm_d = nc.dram_tensor("mmat", [HEADS, C], f32, kind="ExternalInput")
    br_d = nc.dram_tensor("brow", [4, C], f32, kind="ExternalInput")
    id_d = nc.dram_tensor("ident", [128, 128], f32, kind="ExternalInput")
    y_d = nc.dram_tensor("y", [N, C], f32, kind="ExternalOutput")

    NG = 4            # flatten groups (8 nb each)
    GNB = NB // NG    # 8 nb per group
    GN = N // NG      # 1024 n per group

    with tile.TileContext(nc) as tc:
        with tc.tile_pool(name="const", bufs=1) as cp:
            sphi = cp.tile([8, N], f16)            # fp16 hi/lo pairs of s+ / s-
            splo = cp.tile([8, N], f16)
            smhi = cp.tile([8, N], f16)
            smlo = cp.tile([8, N], f16)
            wts_all = cp.tile([8, NB, 128], f32)   # r = num/den, head-major
            wqssb = cp.tile([128, 2, HEADS], f32)
            wkvsb = cp.tile([128, 2, 2], f32)
            mmsb = cp.tile([HEADS, C], f32)
            mmsb16 = cp.tile([HEADS, C], f16)
            bsr_r = cp.tile([1, C], f32)
            gam_r = cp.tile([1, C], f32)
            bet_r = cp.tile([1, C], f32)
            bpr_r = cp.tile([1, C], f32)
            idsb = cp.tile([128, 128], f32)
            ones_row = cp.tile([1, 128], f32)
            ones256 = cp.tile([1, N2], f32)
            eps_sb = cp.tile([128, 1], f32)
            xcv = cp.tile([128, 2, N2], f32)
            xm = cp.tile([128, 2, N2], f32)
            xn = cp.tile([128, 2, N2], f32)
            xnT = cp.tile([128, 2, N2], f32)
            ks_r = cp.tile([1, N2], f32)
            vs_r = cp.tile([1, N2], f32)
            kA_r = cp.tile([1, N2], f32)
            kB_r = cp.tile([1, N2], f32)
            khl16 = cp.tile([1, 4, N2], f16)       # fp16 hi/lo of kA, kB
            khl32 = cp.tile([1, 4, N2], f32)       # same values, f32 for transpose
            kmrow = cp.tile([1, 2, N2], f32)       # kmax row, kmin row
            kcols = cp.tile([128, 12], f32)
            lhsT6 = cp.tile([6, 2, 128], f16)
            bsrep = cp.tile([128, N2], f32)
            garep = cp.tile([128, N2], f32)
            berep = cp.tile([128, N2], f32)
            bprep = cp.tile([128, C], f32)
            kmx = cp.tile([1, 1], f32)
            kmn = cp.tile([1, 1], f32)
            mu = cp.tile([128, 2], f32)
            var = cp.tile([128, 2], f32)
            logv = cp.tile([128, 2], f32)
            rstd = cp.tile([128, 2], f32)
            sq = cp.tile([128, N2], f32)
            vo0 = cp.tile([128, 2], bf16)          # [vs | 1] reduction weights
            vo1 = cp.tile([128, 2], bf16)

            nc.sync.dma_start(idsb[:], id_d[:])
            nc.vector.memset(ones_row[:], 1.0)
            nc.vector.memset(ones256[:], 1.0)
            nc.vector.memset(eps_sb[:], EPS)
            nc.vector.memset(vo0[:, 1:2], 1.0)
            nc.vector.memset(vo1[:, 1:2], 1.0)

            # ---------------- Phase A/B/C: transposes, sT, conv, LN, ks/vs ---
            with tc.tile_pool(name="wsp", bufs=1) as wsp:
                wssb = wsp.tile([128, 32, C], bf16)
                xT = wsp.tile([128, 2, N], f32)    # x transposed, c on partitions
                xTbf = wsp.tile([128, 2, N], bf16)
                with tc.tile_pool(name="xsp1", bufs=1) as xsp1:
                    xs = xsp1.tile([128, NB, C], f32)
                    xs4 = xs[:].rearrange("p (f t) c -> p f t c", f=4)
                    xd4 = x_d[:].rearrange("(f t p) c -> p f t c", p=128, f=4)
                    for f in range(4):
                        nc.sync.dma_start(xs4[:, f, :, :], xd4[:, f, :, :])
                    # weights after x on the queue; Wsr cast to bf16 via gpsimd
                    nc.sync.dma_start(wqssb[:], wqs_d[:].rearrange("(t p) h -> p t h", p=128))
                    nc.sync.dma_start(wkvsb[:], wkv_d[:].rearrange("(t p) h -> p t h", p=128))
                    nc.sync.dma_start(mmsb[:], mm_d[:])
                    nc.sync.dma_start(bsr_r[:], br_d[0:1, :])
                    nc.sync.dma_start(gam_r[:], br_d[1:2, :])
                    nc.sync.dma_start(bet_r[:], br_d[2:3, :])
                    nc.sync.dma_start(bpr_r[:], br_d[3:4, :])
                    nc.gpsimd.dma_start(wssb[:], ws_d[:].rearrange("(t p) c -> p t c", p=128))
                    nc.vector.tensor_copy(mmsb16[:], mmsb[:])

                    with tc.tile_pool(name="psA", bufs=2, space="PSUM") as pA:
                        for g in range(32):
                            tp = pA.tile([128, 2, 128], f32)
                            ct, t0 = divmod(g * 2, NB)
                            for q in range(2):
                                nc.tensor.transpose(
                                    tp[:, q, :], xs[:, t0 + q, 128 * ct:128 * (ct + 1)],
                                    idsb[:],
                                )
                            v2 = xT[:, ct, :].rearrange(
                                "p (kh kw ph pw) -> p kh kw ph pw", kh=4, kw=4, ph=16, pw=16
                            )
                            dst = v2[:, :, :, t0 // 2, :].rearrange("p kh kw pw -> p kh pw kw")
                            srcv = tp[:].rearrange(
                                "p q i -> p (q i)"
                            ).rearrange("p (kha pw kw) -> p kha pw kw", kha=4, pw=16, kw=4)
                            eng = nc.vector.tensor_copy if g % 2 == 0 else nc.scalar.copy
                            eng(dst, srcv)

                nc.vector.tensor_copy(xTbf[:], xT[:])

                with tc.tile_pool(name="xsp2", bufs=1) as xsp2:
                    sT_sb = xsp2.tile([8, N], f32)
                    zeros8 = xsp2.tile([8, N], f32)
                    scratch = xsp2.tile([8, N], f32)
                    nc.vector.memset(zeros8[:], 0.0)
                    with tc.tile_pool(name="psA2", bufs=2, space="PSUM") as pA2:
                        # sT = wqs^T @ xT  (h on partitions, n free, pos order)
                        for k in range(8):
                            sps = pA2.tile([8, 512], f32)
                            for ct in range(2):
                                nc.tensor.matmul(
                                    sps[:],
                                    lhsT=wqssb[:, ct, :],
                                    rhs=xT[:, ct, 512 * k:512 * (k + 1)],
                                    start=(ct == 0),
                                    stop=(ct == 1),
                                )
                            eng = nc.vector.tensor_copy if k % 2 == 0 else nc.scalar.copy
                            eng(sT_sb[:, 512 * k:512 * (k + 1)], sps[:])
                    # s+/s- fp16 hi/lo splits
                    nc.vector.tensor_tensor(scratch[:], sT_sb[:], zeros8[:], ALU.max)
                    nc.vector.tensor_copy(sphi[:], scratch[:])
                    nc.vector.tensor_tensor(splo[:], scratch[:], sphi[:], ALU.subtract)
                    nc.vector.tensor_tensor(scratch[:], sT_sb[:], zeros8[:], ALU.min)
                    nc.vector.tensor_copy(smhi[:], scratch[:])
                    nc.vector.tensor_tensor(smlo[:], scratch[:], smhi[:], ALU.subtract)

                # conv (kernel=stride=4) + LayerNorm
                with tc.tile_pool(name="psB", bufs=2, space="PSUM") as pB:
                    brp = pB.tile([128, N2], f32, bufs=1)
                    nc.tensor.matmul(brp[:], lhsT=ones_row[:], rhs=bsr_r[:],
                                     start=True, stop=True)
                    nc.vector.tensor_copy(bsrep[:], brp[:])
                    grp = pB.tile([128, N2], f32, bufs=1)
                    nc.tensor.matmul(grp[:], lhsT=ones_row[:], rhs=gam_r[:],
                                     start=True, stop=True)
                    nc.vector.tensor_copy(garep[:], grp[:])
                    bep = pB.tile([128, N2], f32, bufs=1)
                    nc.tensor.matmul(bep[:], lhsT=ones_row[:], rhs=bet_r[:],
                                     start=True, stop=True)
                    nc.vector.tensor_copy(berep[:], bep[:])
                    bpp = pB.tile([128, C], f32, bufs=1)
                    nc.tensor.matmul(bpp[:], lhsT=ones_row[:], rhs=bpr_r[:],
                                     start=True, stop=True)
                    nc.vector.tensor_copy(bprep[:], bpp[:])
                    for m in range(2):
                        cps = pB.tile([128, N2], f32)
                        for kh in range(4):
                            for kw in range(4):
                                for ct in range(2):
                                    kidx = kh * 8 + kw * 2 + ct
                                    base = (kh * 4 + kw) * 256 + 128 * m
                                    nc.tensor.matmul(
                                        cps[:],
                                        lhsT=xTbf[:, ct, base:base + 128],
                                        rhs=wssb[:, kidx, :],
                                        start=(kidx == 0),
                                        stop=(kidx == 31),
                                    )
                        nc.vector.tensor_tensor(xcv[:, m, :], cps[:], bsrep[:], ALU.add)

                    for m in range(2):
                        nc.vector.reduce_sum(mu[:, m:m + 1], xcv[:, m, :], axis=AX.X)
                        nc.vector.tensor_scalar(
                            mu[:, m:m + 1], mu[:, m:m + 1], 1.0 / N2, None, ALU.mult
                        )
                        nc.vector.tensor_scalar(
                            xm[:, m, :], xcv[:, m, :], mu[:, m:m + 1], None, ALU.subtract
                        )
                        nc.vector.tensor_tensor(sq[:], xm[:, m, :], xm[:, m, :],
                                                ALU.mult)
                        nc.vector.reduce_sum(var[:, m:m + 1], sq[:], axis=AX.X)
                        nc.scalar.activation(
                            logv[:, m:m + 1], var[:, m:m + 1], AF.Ln,
                            bias=eps_sb[:], scale=1.0 / N2,
                        )
                        nc.scalar.activation(
                            rstd[:, m:m + 1], logv[:, m:m + 1], AF.Exp, scale=-0.5
                        )
                        nc.vector.tensor_scalar(
                            xn[:, m, :], xm[:, m, :], rstd[:, m:m + 1], None, ALU.mult
                        )
                        nc.vector.tensor_tensor(xm[:, m, :], xn[:, m, :], garep[:], ALU.mult)
                        nc.vector.tensor_tensor(xn[:, m, :], xm[:, m, :], berep[:], ALU.add)

            # ks/vs rows, k split-precision prep
            with tc.tile_pool(name="psC", bufs=1, space="PSUM") as pC:
                tp2 = pC.tile([128, 4, 128], f32)
                for t2 in range(2):
                    for ct in range(2):
                        nc.tensor.transpose(
                            tp2[:, t2 * 2 + ct, :],
                            xn[:, t2, 128 * ct:128 * (ct + 1)],
                            idsb[:],
                        )
                for t2 in range(2):
                    for ct in range(2):
                        nc.vector.tensor_copy(
                            xnT[:, ct, 128 * t2:128 * (t2 + 1)], tp2[:, t2 * 2 + ct, :]
                        )
                kps_k = pC.tile([1, N2], f32)
                kps_v = pC.tile([1, N2], f32)
                for ct in range(2):
                    nc.tensor.matmul(
                        kps_k[:], lhsT=wkvsb[:, ct, 0:1], rhs=xnT[:, ct, :],
                        start=(ct == 0), stop=(ct == 1),
                    )
                    nc.tensor.matmul(
                        kps_v[:], lhsT=wkvsb[:, ct, 1:2], rhs=xnT[:, ct, :],
                        start=(ct == 0), stop=(ct == 1),
                    )
                nc.vector.tensor_copy(ks_r[:], kps_k[:])
                nc.vector.tensor_copy(vs_r[:], kps_v[:])
                nc.vector.reduce_max(kmx[:], ks_r[:], axis=AX.X)
                nc.vector.tensor_reduce(kmn[:], ks_r[:], axis=AX.X, op=ALU.min)
                # kmax/kmin broadcast rows; kA = ks - kmax, kB = ks - kmin
                kmp = pC.tile([1, 2, N2], f32)
                nc.tensor.matmul(kmp[:, 0, :], lhsT=kmx[:], rhs=ones256[:],
                                 start=True, stop=True)
                nc.tensor.matmul(kmp[:, 1, :], lhsT=kmn[:], rhs=ones256[:],
                                 start=True, stop=True)
                nc.vector.tensor_copy(kmrow[:], kmp[:])
                nc.vector.tensor_tensor(kA_r[:], ks_r[:], kmrow[:, 0, :], ALU.subtract)
                nc.vector.tensor_tensor(kB_r[:], ks_r[:], kmrow[:, 1, :], ALU.subtract)
                # fp16 hi/lo split of kA/kB, kept in f32 rows for PE transpose
                nc.vector.tensor_copy(khl16[:, 0, :], kA_r[:])
                nc.vector.tensor_copy(khl32[:, 0, :], khl16[:, 0, :])
                nc.vector.tensor_tensor(khl16[:, 1, :], kA_r[:], khl32[:, 0, :],
                                        ALU.subtract)
                nc.vector.tensor_copy(khl32[:, 1, :], khl16[:, 1, :])
                nc.vector.tensor_copy(khl16[:, 2, :], kB_r[:])
                nc.vector.tensor_copy(khl32[:, 2, :], khl16[:, 2, :])
                nc.vector.tensor_tensor(khl16[:, 3, :], kB_r[:], khl32[:, 2, :],
                                        ALU.subtract)
                nc.vector.tensor_copy(khl32[:, 3, :], khl16[:, 3, :])
                # pack k columns per jt with duplicates, transpose to lhsT6
                # lhsT6 rows: (kAhi, kAhi, kAlo, kBhi, kBhi, kBlo)
                kcp = pC.tile([128, 12], f32)
                for jt in range(2):
                    for ci, ki in ((0, 0), (1, 0), (2, 1), (3, 2), (4, 2), (5, 3)):
                        nc.tensor.transpose(
                            kcp[:, 6 * jt + ci:6 * jt + ci + 1],
                            khl32[0:1, ki, 128 * jt:128 * (jt + 1)], idsb[0:1, 0:1],
                        )
                nc.vector.tensor_copy(kcols[:], kcp[:])
                l6p = pC.tile([6, 2, 128], f32)
                for jt in range(2):
                    nc.tensor.transpose(l6p[:, jt, :], kcols[:, 6 * jt:6 * jt + 6],
                                        idsb[:])
                nc.vector.tensor_copy(lhsT6[:], l6p[:])
                # vs as columns
                tp3 = pC.tile([128, 2], f32)
                nc.tensor.transpose(tp3[:, 0:1], vs_r[0:1, 0:128], idsb[0:1, 0:1])
                nc.tensor.transpose(tp3[:, 1:2], vs_r[0:1, 128:256], idsb[0:1, 0:1])
                nc.vector.tensor_copy(vo0[:, 0:1], tp3[:, 0:1])
                nc.vector.tensor_copy(vo1[:, 0:1], tp3[:, 1:2])

            # ------------- Main: flatten rows, X -> Exp -> num/den -> wts -> y
            y5 = y_d[:].rearrange(
                "(ph q pw r) c -> ph q pw r c", ph=16, q=4, pw=16, r=4
            )
            with (
                tc.tile_pool(name="stp", bufs=2) as stp,
                tc.tile_pool(name="ep", bufs=4) as ep,
                tc.tile_pool(name="ndp", bufs=2) as ndp,
                tc.tile_pool(name="ntp", bufs=2) as ntp,
                tc.tile_pool(name="dtp", bufs=2) as dtp,
                tc.tile_pool(name="dvp", bufs=2) as dvp,
                tc.tile_pool(name="wtp", bufs=2) as wtp,
                tc.tile_pool(name="ysq", bufs=2) as ysq,
                tc.tile_pool(name="pX", bufs=2, space="PSUM") as pX,
                tc.tile_pool(name="pND", bufs=1, space="PSUM") as pND,
                tc.tile_pool(name="pY", bufs=2, space="PSUM") as pY,
            ):
                st_list = {}

                def issue_group(g):
                    gs, ge = GN * g, GN * (g + 1)
                    ST = stp.tile([6, 8, GNB, 128], f16, name=f"ST{g}", tag="ST")
                    for r, src in ((0, sphi), (1, splo), (2, sphi),
                                   (3, smhi), (4, smlo), (5, smhi)):
                        srcv = src[:, gs:ge].rearrange("p (nb n) -> p nb n", nb=GNB)
                        nc.sync.dma_start(ST[r:r + 1], srcv)
                    st_list[g] = ST

                issue_group(0)

                e_pair = [None, None]
                nd4 = None
                for c in range(2 * NB):
                    nb, jt = divmod(c, 2)
                    g, nbl = divmod(nb, GNB)
                    q4, nb4 = divmod(nb, 4)       # 4-nb division batches
                    if nbl == 0 and jt == 0 and g + 1 < NG:
                        issue_group(g + 1)
                    Xp = pX.tile([128, 1024], f32)
                    for k in range(2):
                        nc.tensor.matmul(
                            Xp[:, 512 * k:512 * (k + 1)],
                            lhsT=lhsT6[:, jt, :],
                            rhs=st_list[g][:, 4 * k:4 * k + 4, nbl, :],
                            start=True, stop=True,
                        )
                    E = ep.tile([128, 1024], bf16)
                    nc.scalar.activation(E[:], Xp[:], AF.Exp)
                    e_pair[jt] = E
                    if jt == 0:
                        continue
                    ND = pND.tile([2, 1024], f32)
                    for k in range(2):
                        sl = slice(512 * k, 512 * (k + 1))
                        nc.tensor.matmul(ND[:, sl], lhsT=vo0[:],
                                         rhs=e_pair[0][:, sl],
                                         start=True, stop=False)
                        nc.tensor.matmul(ND[:, sl], lhsT=vo1[:],
                                         rhs=e_pair[1][:, sl],
                                         start=False, stop=True)
                    if nb4 == 0:
                        nd4 = ndp.tile([2, 8, 4, 128], f16, name=f"nd4_{q4}",
                                       tag="nd4")
                    nc.vector.tensor_copy(
                        nd4[:, :, nb4, :],
                        ND[:].rearrange("p (h n) -> p h n", h=8))
                    if nb4 != 3:
                        continue
                    # batched division for nb = 4*q4 .. 4*q4+3
                    numT = ntp.tile([8, 4, 128], f16)
                    denT = dtp.tile([8, 4, 128], f16)
                    nc.sync.dma_start(numT[:], nd4[0:1])
                    nc.sync.dma_start(denT[:], nd4[1:2])
                    dinvT = dvp.tile([8, 4, 128], f16)
                    with nc.allow_low_precision(reason="den in [1,256]; fp16 rel 5e-4 ok"):
                        nc.vector.reciprocal(dinvT[:], denT[:])
                    wts4 = wtp.tile([8, 4, 128], f16)
                    nc.vector.tensor_tensor(wts4[:], numT[:], dinvT[:], ALU.mult)
                    for i4 in range(4):
                        nbq = 4 * q4 + i4
                        yp = pY.tile([128, C], f32)
                        nc.tensor.matmul(yp[:], lhsT=wts4[:, i4, :], rhs=mmsb16[:],
                                         start=True, stop=True)
                        ysb = ysq.tile([128, C], f32)
                        nc.vector.tensor_tensor(ysb[:], yp[:], bprep[:], ALU.add)
                        kh, kw = divmod(nbq // 2, 4)
                        nc.gpsimd.dma_start(
                            y5[8 * (nbq % 2):8 * (nbq % 2) + 8, kh, :, kw, :], ysb[:]
                        )

    nc.compile()
    return nc


def _host_precompute(Wq, Wkv, Wsr, bsr, gamma, beta, Wproj, bproj, k_learn, v_learn):
    lksum = k_learn.reshape(HEADS, HC).sum(1)
    wqs = (Wq.reshape(C, HEADS, HC).sum(2) * (SCALE * lksum)[None, :]).astype(np.float32)
    wkv2 = np.stack([Wkv[:, 0::2].sum(1), Wkv[:, 1::2].sum(1)], 1).astype(np.float32)
    lv = v_learn.reshape(HEADS, HC)
    # out rearrange 'b d n c -> b n (c d)': column index = ci*HEADS + h
    Mmat = np.zeros((HEADS, C), np.float32)
    for h in range(HEADS):
        Mmat[h] = lv[h] @ Wproj[h::HEADS]
    Wsr_flat = np.ascontiguousarray(
        Wsr.transpose(2, 3, 1, 0).reshape(SR * SR * C, C)
    ).astype(np.float32)
    brow = np.stack([bsr, gamma, beta, bproj]).astype(np.float32)
    ident = np.eye(128, dtype=np.float32)
    return dict(wsr=Wsr_flat, wqs=wqs, wkv2=wkv2, mmat=Mmat, brow=brow, ident=ident)


def kernel(**inputs):
    x = np.asarray(inputs["x"], np.float32)
    weights = _host_precompute(
        *[np.asarray(inputs[k], np.float32) for k in
          ("Wq", "Wkv", "Wsr", "bsr", "gamma", "beta", "Wproj", "bproj",
           "k_learn", "v_learn")]
    )
    if "nc" not in _NC_CACHE:
        _NC_CACHE["nc"] = _build_nc()
    nc = _NC_CACHE["nc"]
    in_maps = [
        {"x": np.ascontiguousarray(x[i]), **weights} for i in range(B)
    ]
    from concourse.bass_utils import run_bass_kernel_spmd

    res = run_bass_kernel_spmd(nc, in_maps, core_ids=list(range(B)))
    y = np.stack([res.results[i]["y"] for i in range(B)], 0)
    return y


# revision 25
# speedup vs baseline: 1.7888x; 1.0834x over previous
"""Trainium2 Bass kernel for nn_Attention_17489106830121.

Math: the reference's einsums sum out entire axes, making attention logits
rank-1: attn[b,h,n,j] = s[b,n,h] * ks[b,j], with
  s  = x @ wqs              (wqs folds Wq head-colsums * SCALE * lksum)
  ks = LN(conv(x)) @ wk     (wk = even-col sums of Wkv)
  vs = LN(conv(x)) @ wv     (odd-col sums)
  out[b,n,:] = (softmax-weighted vs means over j, per (n,h)) @ Mmat + bproj

Per core: one batch element (8 cores == B).

v3: j-on-partitions attention middle, minimal PE instruction count.
Shifted logits via sign-split (exact, no shift row):
  X[j, (n,h)] = (ks_j - kmax) * s+  +  (ks_j - kmin) * s-
with s+/s- and the two k-vectors in fp16 hi/lo pairs (K=6 rank per chunk,
error ~1e-4). One matmul per (nb, jt) builds X [128, 1024]; ScalarE Exp's
it psum->sbuf bf16; one [vs|1]-weighted matmul per jt reduces num/den to
[2, 1024] rows; DMA reshapes rows to [8, 128] head-major tiles where the
division runs on DVE; final y = wts.T @ Mmat + bias.
"""

import numpy as np

B, N, C, HEADS, SR = 8, 4096, 256, 8, 4
HC = C // HEADS          # 32
SCALE = HC ** -0.5
EPS = 1e-5
HS = 64 // SR            # 16
N2 = HS * HS             # 256
NB = N // 128            # 32 row tiles

_NC_CACHE = {}


def _build_nc():
    import concourse.bass as bass
    import concourse.bacc as bacc
    import concourse.mybir as mybir
    from concourse import tile

    dt = mybir.dt
    f32, bf16, f16 = dt.float32, dt.bfloat16, dt.float16
    AF = mybir.ActivationFunctionType
    ALU = mybir.AluOpType
    AX = mybir.AxisListType

    nc = bacc.Bacc(None, target_bir_lowering=False)

    x_d = nc.dram_tensor("x", [N, C], f32, kind="ExternalInput")
    ws_d = nc.dram_tensor("wsr", [SR * SR * C, C], f32, kind="ExternalInput")
    wqs_d = nc.dram_tensor("wqs", [C, HEADS], f32, kind="ExternalInput")
    wkv_d = nc.dram_tensor("wkv2", [C, 2], f32, kind="ExternalInput")
    mm_d = nc.dram_tensor("mmat", [HEADS, C], f32, kind="ExternalInput")
    br_d = nc.dram_tensor("brow", [4, C], f32, kind="ExternalInput")
    id_d = nc.dram_tensor("ident", [128, 128], f32, kind="ExternalInput")
    y_d = nc.dram_tensor("y", [N, C], f32, kind="ExternalOutput")

    NG = 4            # flatten groups (8 nb each)
    GNB = NB // NG    # 8 nb per group
    GN = N // NG      # 1024 n per group

    with tile.TileContext(nc) as tc:
        with tc.tile_pool(name="const", bufs=1) as cp:
            sphi = cp.tile([8, N], f16)            # fp16 hi/lo pairs of s+ / s-
            splo = cp.tile([8, N], f16)
            smhi = cp.tile([8, N], f16)
            smlo = cp.tile([8, N], f16)
            wts_all = cp.tile([8, NB, 128], f32)   # r = num/den, head-major
            wqssb = cp.tile([128, 2, HEADS], f32)
            wkvsb = cp.tile([128, 2, 2], f32)
            mmsb = cp.tile([HEADS, C], f32)
            mmsb16 = cp.tile([HEADS, C], f16)
            bsr_r = cp.tile([1, C], f32)
            gam_r = cp.tile([1, C], f32)
            bet_r = cp.tile([1, C], f32)
            bpr_r = cp.tile([1, C], f32)
            idsb = cp.tile([128, 128], f32)
            ones_row = cp.tile([1, 128], f32)
            ones256 = cp.tile([1, N2], f32)
            eps_sb = cp.tile([128, 1], f32)
            xcv = cp.tile([128, 2, N2], f32)
            xm = cp.tile([128, 2, N2], f32)
            xn = cp.tile([128, 2, N2], f32)
            xnT = cp.tile([128, 2, N2], f32)
            ks_r = cp.tile([1, N2], f32)
            vs_r = cp.tile([1, N2], f32)
            kA_r = cp.tile([1, N2], f32)
            kB_r = cp.tile([1, N2], f32)
            khl16 = cp.tile([1, 4, N2], f16)       # fp16 hi/lo of kA, kB
            khl32 = cp.tile([1, 4, N2], f32)       # same values, f32 for transpose
            kmrow = cp.tile([1, 2, N2], f32)       # kmax row, kmin row
            kcols = cp.tile([128, 12], f32)
            lhsT6 = cp.tile([6, 2, 128], f16)
            bsrep = cp.tile([128, N2], f32)
            garep = cp.tile([128, N2], f32)
            berep = cp.tile([128, N2], f32)
            bprep = cp.tile([128, C], f32)
            kmx = cp.tile([1, 1], f32)
            kmn = cp.tile([1, 1], f32)
            mu = cp.tile([128, 2], f32)
            var = cp.tile([128, 2], f32)
            logv = cp.tile([128, 2], f32)
            rstd = cp.tile([128, 2], f32)
            sq = cp.tile([128, N2], f32)
            vo0 = cp.tile([128, 2], bf16)          # [vs | 1] reduction weights
            vo1 = cp.tile([128, 2], bf16)

            nc.sync.dma_start(idsb[:], id_d[:])
            nc.vector.memset(ones_row[:], 1.0)
            nc.vector.memset(ones256[:], 1.0)
            nc.vector.memset(eps_sb[:], EPS)
            nc.vector.memset(vo0[:, 1:2], 1.0)
            nc.vector.memset(vo1[:, 1:2], 1.0)

            # ---------------- Phase A/B/C: transposes, sT, conv, LN, ks/vs ---
            with tc.tile_pool(name="wsp", bufs=1) as wsp:
                wssb = wsp.tile([128, 32, C], bf16)
                xT = wsp.tile([128, 2, N], f32)    # x transposed, c on partitions
                xTbf = wsp.tile([128, 2, N], bf16)
                with tc.tile_pool(name="xsp1", bufs=1) as xsp1:
                    xs = xsp1.tile([128, NB, C], f32)
                    xs4 = xs[:].rearrange("p (f t) c -> p f t c", f=4)
                    xd4 = x_d[:].rearrange("(f t p) c -> p f t c", p=128, f=4)
                    for f in range(4):
                        nc.sync.dma_start(xs4[:, f, :, :], xd4[:, f, :, :])
                    # weights after x on the queue; Wsr cast to bf16 via gpsimd
                    nc.sync.dma_start(wqssb[:], wqs_d[:].rearrange("(t p) h -> p t h", p=128))
                    nc.sync.dma_start(wkvsb[:], wkv_d[:].rearrange("(t p) h -> p t h", p=128))
                    nc.sync.dma_start(mmsb[:], mm_d[:])
                    nc.sync.dma_start(bsr_r[:], br_d[0:1, :])
                    nc.sync.dma_start(gam_r[:], br_d[1:2, :])
                    nc.sync.dma_start(bet_r[:], br_d[2:3, :])
                    nc.sync.dma_start(bpr_r[:], br_d[3:4, :])
                    nc.gpsimd.dma_start(wssb[:], ws_d[:].rearrange("(t p) c -> p t c", p=128))
                    nc.vector.tensor_copy(mmsb16[:], mmsb[:])

                    with tc.tile_pool(name="psA", bufs=2, space="PSUM") as pA:
                        for g in range(32):
                            tp = pA.tile([128, 2, 128], f32)
                            ct, t0 = divmod(g * 2, NB)
                            for q in range(2):
                                nc.tensor.transpose(
                                    tp[:, q, :], xs[:, t0 + q, 128 * ct:128 * (ct + 1)],
                                    idsb[:],
                                )
                            v2 = xT[:, ct, :].rearrange(
                                "p (kh kw ph pw) -> p kh kw ph pw", kh=4, kw=4, ph=16, pw=16
                            )
                            dst = v2[:, :, :, t0 // 2, :].rearrange("p kh kw pw -> p kh pw kw")
                            srcv = tp[:].rearrange(
                                "p q i -> p (q i)"
                            ).rearrange("p (kha pw kw) -> p kha pw kw", kha=4, pw=16, kw=4)
                            eng = nc.vector.tensor_copy if g % 2 == 0 else nc.scalar.copy
                            eng(dst, srcv)

                nc.vector.tensor_copy(xTbf[:], xT[:])

                with tc.tile_pool(name="xsp2", bufs=1) as xsp2:
                    sT_sb = xsp2.tile([8, N], f32)
                    zeros8 = xsp2.tile([8, N], f32)
                    scratch = xsp2.tile([8, N], f32)
                    nc.vector.memset(zeros8[:], 0.0)
                    with tc.tile_pool(name="psA2", bufs=2, space="PSUM") as pA2:
                        # sT = wqs^T @ xT  (h on partitions, n free, pos order)
                        for k in range(8):
                            sps = pA2.tile([8, 512], f32)
                            for ct in range(2):
                                nc.tensor.matmul(
                                    sps[:],
                                    lhsT=wqssb[:, ct, :],
                                    rhs=xT[:, ct, 512 * k:512 * (k + 1)],
                                    start=(ct == 0),
                                    stop=(ct == 1),
                                )
                            eng = nc.vector.tensor_copy if k % 2 == 0 else nc.scalar.copy
                            eng(sT_sb[:, 512 * k:512 * (k + 1)], sps[:])
                    # s+/s- fp16 hi/lo splits
                    nc.vector.tensor_tensor(scratch[:], sT_sb[:], zeros8[:], ALU.max)
                    nc.vector.tensor_copy(sphi[:], scratch[:])
                    nc.vector.tensor_tensor(splo[:], scratch[:], sphi[:], ALU.subtract)
                    nc.vector.tensor_tensor(scratch[:], sT_sb[:], zeros8[:], ALU.min)
                    nc.vector.tensor_copy(smhi[:], scratch[:])
                    nc.vector.tensor_tensor(smlo[:], scratch[:], smhi[:], ALU.subtract)

                # conv (kernel=stride=4) + LayerNorm
                with tc.tile_pool(name="psB", bufs=2, space="PSUM") as pB:
                    brp = pB.tile([128, N2], f32, bufs=1)
                    nc.tensor.matmul(brp[:], lhsT=ones_row[:], rhs=bsr_r[:],
                                     start=True, stop=True)
                    nc.vector.tensor_copy(bsrep[:], brp[:])
                    grp = pB.tile([128, N2], f32, bufs=1)
                    nc.tensor.matmul(grp[:], lhsT=ones_row[:], rhs=gam_r[:],
                                     start=True, stop=True)
                    nc.vector.tensor_copy(garep[:], grp[:])
                    bep = pB.tile([128, N2], f32, bufs=1)
                    nc.tensor.matmul(bep[:], lhsT=ones_row[:], rhs=bet_r[:],
                                     start=True, stop=True)
                    nc.vector.tensor_copy(berep[:], bep[:])
                    bpp = pB.tile([128, C], f32, bufs=1)
                    nc.tensor.matmul(bpp[:], lhsT=ones_row[:], rhs=bpr_r[:],
                                     start=True, stop=True)
                    nc.vector.tensor_copy(bprep[:], bpp[:])
                    for m in range(2):
                        cps = pB.tile([128, N2], f32)
                        for kh in range(4):
                            for kw in range(4):
                                for ct in range(2):
                                    kidx = kh * 8 + kw * 2 + ct
                                    base = (kh * 4 + kw) * 256 + 128 * m
                                    nc.tensor.matmul(
                                        cps[:],
                                        lhsT=xTbf[:, ct, base:base + 128],
                                        rhs=wssb[:, kidx, :],
                                        start=(kidx == 0),
                                        stop=(kidx == 31),
                                    )
                        nc.vector.tensor_tensor(xcv[:, m, :], cps[:], bsrep[:], ALU.add)

                    for m in range(2):
                        nc.vector.reduce_sum(mu[:, m:m + 1], xcv[:, m, :], axis=AX.X)
                        nc.vector.tensor_scalar(
                            mu[:, m:m + 1], mu[:, m:m + 1], 1.0 / N2, None, ALU.mult
                        )
                        nc.vector.tensor_scalar(
                            xm[:, m, :], xcv[:, m, :], mu[:, m:m + 1], None, ALU.subtract
                        )
                        nc.vector.tensor_tensor(sq[:], xm[:, m, :], xm[:, m, :],
                                                ALU.mult)
                        nc.vector.reduce_sum(var[:, m:m + 1], sq[:], axis=AX.X)
                        nc.scalar.activation(
                            logv[:, m:m + 1], var[:, m:m + 1], AF.Ln,
                            bias=eps_sb[:], scale=1.0 / N2,
                        )
                        nc.scalar.activation(
                            rstd[:, m:m + 1], logv[:, m:m + 1], AF.Exp, scale=-0.5
                        )
                        nc.vector.tensor_scalar(
                            xn[:, m, :], xm[:, m, :], rstd[:, m:m + 1], None, ALU.mult
                        )
                        nc.vector.tensor_tensor(xm[:, m, :], xn[:, m, :], garep[:], ALU.mult)
                        nc.vector.tensor_tensor(xn[:, m, :], xm[:, m, :], berep[:], ALU.add)

            # ks/vs rows, k split-precision prep
            with tc.tile_pool(name="psC", bufs=1, space="PSUM") as pC:
                tp2 = pC.tile([128, 4, 128], f32)
                for t2 in range(2):
                    for ct in range(2):
                        nc.tensor.transpose(
                            tp2[:, t2 * 2 + ct, :],
                            xn[:, t2, 128 * ct:128 * (ct + 1)],
                            idsb[:],
                        )
                for t2 in range(2):
                    for ct in range(2):
                        nc.vector.tensor_copy(
                            xnT[:, ct, 128 * t2:128 * (t2 + 1)], tp2[:, t2 * 2 + ct, :]
                        )
                kps_k = pC.tile([1, N2], f32)
                kps_v = pC.tile([1, N2], f32)
                for ct in range(2):
                    nc.tensor.matmul(
                        kps_k[:], lhsT=wkvsb[:, ct, 0:1], rhs=xnT[:, ct, :],
                        start=(ct == 0), stop=(ct == 1),
                    )
                    nc.tensor.matmul(
                        kps_v[:], lhsT=wkvsb[:, ct, 1:2], rhs=xnT[:, ct, :],
                        start=(ct == 0), stop=(ct == 1),
                    )
                nc.vector.tensor_copy(ks_r[:], kps_k[:])
                nc.vector.tensor_copy(vs_r[:], kps_v[:])
                nc.vector.reduce_max(kmx[:], ks_r[:], axis=AX.X)
                nc.vector.tensor_reduce(kmn[:], ks_r[:], axis=AX.X, op=ALU.min)
                # kmax/kmin broadcast rows; kA = ks - kmax, kB = ks - kmin
                kmp = pC.tile([1, 2, N2], f32)
                nc.tensor.matmul(kmp[:, 0, :], lhsT=kmx[:], rhs=ones256[:],
                                 start=True, stop=True)
                nc.tensor.matmul(kmp[:, 1, :], lhsT=kmn[:], rhs=ones256[:],
                                 start=True, stop=True)
                nc.vector.tensor_copy(kmrow[:], kmp[:])
                nc.vector.tensor_tensor(kA_r[:], ks_r[:], kmrow[:, 0, :], ALU.subtract)
                nc.vector.tensor_tensor(kB_r[:], ks_r[:], kmrow[:, 1, :], ALU.subtract)
                # fp16 hi/lo split of kA/kB, kept in f32 rows for PE transpose
                nc.vector.tensor_copy(khl16[:, 0, :], kA_r[:])
                nc.vector.tensor_copy(khl32[:, 0, :], khl16[:, 0, :])
                nc.vector.tensor_tensor(khl16[:, 1, :], kA_r[:], khl32[:, 0, :],
                                        ALU.subtract)
                nc.vector.tensor_copy(khl32[:, 1, :], khl16[:, 1, :])
                nc.vector.tensor_copy(khl16[:, 2, :], kB_r[:])
                nc.vector.tensor_copy(khl32[:, 2, :], khl16[:, 2, :])
                nc.vector.tensor_tensor(khl16[:, 3, :], kB_r[:], khl32[:, 2, :],
                                        ALU.subtract)
                nc.vector.tensor_copy(khl32[:, 3, :], khl16[:, 3, :])
                # pack k columns per jt with duplicates, transpose to lhsT6
                # lhsT6 rows: (kAhi, kAhi, kAlo, kBhi, kBhi, kBlo)
                kcp = pC.tile([128, 12], f32)
                for jt in range(2):
                    for ci, ki in ((0, 0), (1, 0), (2, 1), (3, 2), (4, 2), (5, 3)):
                        nc.tensor.transpose(
                            kcp[:, 6 * jt + ci:6 * jt + ci + 1],
                            khl32[0:1, ki, 128 * jt:128 * (jt + 1)], idsb[0:1, 0:1],
                        )
                nc.vector.tensor_copy(kcols[:], kcp[:])
                l6p = pC.tile([6, 2, 128], f32)
                for jt in range(2):
                    nc.tensor.transpose(l6p[:, jt, :], kcols[:, 6 * jt:6 * jt + 6],
                                        idsb[:])
                nc.vector.tensor_copy(lhsT6[:], l6p[:])
                # vs as columns
                tp3 = pC.tile([128, 2], f32)
                nc.tensor.transpose(tp3[:, 0:1], vs_r[0:1, 0:128], idsb[0:1, 0:1])
                nc.tensor.transpose(tp3[:, 1:2], vs_r[0:1, 128:256], idsb[0:1, 0:1])
                nc.vector.tensor_copy(vo0[:, 0:1], tp3[:, 0:1])
                nc.vector.tensor_copy(vo1[:, 0:1], tp3[:, 1:2])

            # ------------- Main: flatten rows, X -> Exp -> num/den -> wts -> y
            y5 = y_d[:].rearrange(
                "(ph q pw r) c -> ph q pw r c", ph=16, q=4, pw=16, r=4
            )
            with (
                tc.tile_pool(name="stp", bufs=2) as stp,
                tc.tile_pool(name="ep", bufs=4) as ep,
                tc.tile_pool(name="ndp", bufs=2) as ndp,
                tc.tile_pool(name="ntp", bufs=2) as ntp,
                tc.tile_pool(name="dtp", bufs=2) as dtp,
                tc.tile_pool(name="dvp", bufs=2) as dvp,
                tc.tile_pool(name="wtp", bufs=2) as wtp,
                tc.tile_pool(name="ysq", bufs=2) as ysq,
                tc.tile_pool(name="pX", bufs=2, space="PSUM") as pX,
                tc.tile_pool(name="pND", bufs=1, space="PSUM") as pND,
                tc.tile_pool(name="pY", bufs=2, space="PSUM") as pY,
            ):
                st_list = {}

                def issue_group(g):
                    gs, ge = GN * g, GN * (g + 1)
                    ST = stp.tile([6, 8, GNB, 128], f16, name=f"ST{g}", tag="ST")
                    for r, src in ((0, sphi), (1, splo), (2, sphi),
                                   (3, smhi), (4, smlo), (5, smhi)):
                        srcv = src[:, gs:ge].rearrange("p (nb n) -> p nb n", nb=GNB)
                        nc.sync.dma_start(ST[r:r + 1], srcv)
                    st_list[g] = ST

                issue_group(0)

                e_pair = [None, None]
                nd4 = None
                for c in range(2 * NB):
                    nb, jt = divmod(c, 2)
                    g, nbl = divmod(nb, GNB)
                    q4, nb4 = divmod(nb, 4)       # 4-nb division batches
                    if nbl == 0 and jt == 0 and g + 1 < NG:
                        issue_group(g + 1)
                    Xp = pX.tile([128, 1024], f32)
                    for k in range(2):
                        nc.tensor.matmul(
                            Xp[:, 512 * k:512 * (k + 1)],
                            lhsT=lhsT6[:, jt, :],
                            rhs=st_list[g][:, 4 * k:4 * k + 4, nbl, :],
                            start=True, stop=True,
                        )
                    E = ep.tile([128, 1024], bf16)
                    nc.scalar.activation(E[:], Xp[:], AF.Exp)
                    e_pair[jt] = E
                    if jt == 0:
                        continue
                    ND = pND.tile([2, 1024], f32)
                    for k in range(2):
                        sl = slice(512 * k, 512 * (k + 1))
                        nc.tensor.matmul(ND[:, sl], lhsT=vo0[:],
                                         rhs=e_pair[0][:, sl],
                                         start=True, stop=False)
                        nc.tensor.matmul(ND[:, sl], lhsT=vo1[:],
                                         rhs=e_pair[1][:, sl],
                                         start=False, stop=True)
                    if nb4 == 0:
                        nd4 = ndp.tile([2, 8, 4, 128], f32, name=f"nd4_{q4}",
                                       tag="nd4")
                    eng = nc.vector.tensor_copy if nb % 2 == 0 else nc.scalar.copy
                    eng(nd4[:, :, nb4, :],
                        ND[:].rearrange("p (h n) -> p h n", h=8))
                    if nb4 != 3:
                        continue
                    # batched division for nb = 4*q4 .. 4*q4+3
                    numT = ntp.tile([8, 4, 128], f32)
                    denT = dtp.tile([8, 4, 128], f32)
                    nc.sync.dma_start(numT[:], nd4[0:1])
                    nc.sync.dma_start(denT[:], nd4[1:2])
                    dinvT = dvp.tile([8, 4, 128], f32)
                    nc.vector.reciprocal_approx_fast(dinvT[:], denT[:])
                    wts4 = wtp.tile([8, 4, 128], f16)
                    nc.vector.tensor_tensor(wts4[:], numT[:], dinvT[:], ALU.mult)
                    for i4 in range(4):
                        nbq = 4 * q4 + i4
                        yp = pY.tile([128, C], f32)
                        nc.tensor.matmul(yp[:], lhsT=wts4[:, i4, :], rhs=mmsb16[:],
                                         start=True, stop=True)
                        ysb = ysq.tile([128, C], f32)
                        nc.vector.tensor_tensor(ysb[:], yp[:], bprep[:], ALU.add)
                        kh, kw = divmod(nbq // 2, 4)
                        nc.gpsimd.dma_start(
                            y5[8 * (nbq % 2):8 * (nbq % 2) + 8, kh, :, kw, :], ysb[:]
                        )

    nc.compile()
    return nc


def _host_precompute(Wq, Wkv, Wsr, bsr, gamma, beta, Wproj, bproj, k_learn, v_learn):
    lksum = k_learn.reshape(HEADS, HC).sum(1)
    wqs = (Wq.reshape(C, HEADS, HC).sum(2) * (SCALE * lksum)[None, :]).astype(np.float32)
    wkv2 = np.stack([Wkv[:, 0::2].sum(1), Wkv[:, 1::2].sum(1)], 1).astype(np.float32)
    lv = v_learn.reshape(HEADS, HC)
    # out rearrange 'b d n c -> b n (c d)': column index = ci*HEADS + h
    Mmat = np.zeros((HEADS, C), np.float32)
    for h in range(HEADS):
        Mmat[h] = lv[h] @ Wproj[h::HEADS]
    Wsr_flat = np.ascontiguousarray(
        Wsr.transpose(2, 3, 1, 0).reshape(SR * SR * C, C)
    ).astype(np.float32)
    brow = np.stack([bsr, gamma, beta, bproj]).astype(np.float32)
    ident = np.eye(128, dtype=np.float32)
    return dict(wsr=Wsr_flat, wqs=wqs, wkv2=wkv2, mmat=Mmat, brow=brow, ident=ident)


def kernel(**inputs):
    x = np.asarray(inputs["x"], np.float32)
    weights = _host_precompute(
        *[np.asarray(inputs[k], np.float32) for k in
          ("Wq", "Wkv", "Wsr", "bsr", "gamma", "beta", "Wproj", "bproj",
           "k_learn", "v_learn")]
    )
    if "nc" not in _NC_CACHE:
        _NC_CACHE["nc"] = _build_nc()
    nc = _NC_CACHE["nc"]
    in_maps = [
        {"x": np.ascontiguousarray(x[i]), **weights} for i in range(B)
    ]
    from concourse.bass_utils import run_bass_kernel_spmd

    res = run_bass_kernel_spmd(nc, in_maps, core_ids=list(range(B)))
    y = np.stack([res.results[i]["y"] for i in range(B)], 0)
    return y
